# revision 5
# baseline (speedup 1.0000x reference)
"""Bahdanau attention kernel for Trainium2, SPMD over 8 NeuronCores.

Problem shapes: features [32, 2048, 1024] f32, hidden [32, 1024] f32,
W1/W2 [1024, 1024], b1/b2 [1024], V [1024, 1], bv [1].

Returns (context_vector [32, 1024] f32, attention_weights [32, 2048, 1] f32).

Sharding: data-parallel over batch B; each of the 8 cores handles 4 batches
end-to-end (no collectives needed).

Per-core pipeline, per batch b (T=2048 split into 4 chunks of 512 rows):
  1. SWDGE cast-DMA loads the F chunk f32->bf16 as FB [128(tp), 4(o), 1024(d)].
  2. PE transposes FB 128x128 blocks -> FT [128(dp), 8(j), 512(t)] (bf16).
  3. projT[u,t] = sum_j W1[dj,u].T @ FT[dj,t]  (bf16 matmul, PSUM f32).
  4. ScalarE tanh(projT + bh[u]) -> scoreT bf16, where bh = hidden@W2 + b1 + b2
     enters as the per-partition activation bias (free).
  5. logits[1,t] = sum_m V[um].T @ scoreT[um,t]  (matmul, M=1).
  6. Batch end: -max, exp (with accum_out sum), reciprocal; weights out;
     exp-weights bounced through DRAM into t-partition layout [128, 16] bf16;
     context[1,d] = sum_t ew[t] * FB[t,d] via matmul, scaled by 1/sum.
  Context phase of batch b is emitted after chunk 0 of batch b+1 so the PE
  never stalls on the softmax latency.
"""

import numpy as np

import concourse.bass as bass
import concourse.mybir as mybir
import concourse.tile as tile
from concourse import bacc
from concourse.bass_utils import run_bass_kernel_spmd
from concourse.masks import make_identity

N_CORES = 8
B_LOC = 4  # batches per core
T = 2048
D = 1024
U = 1024
CHUNK = 512  # t rows per chunk
N_CHUNKS = T // CHUNK  # 4
O_PER_CHUNK = CHUNK // 128  # 4 t-subtiles per chunk
NJ = D // 128  # 8 d-tiles
NM = U // 128  # 8 u-tiles

F32 = mybir.dt.float32
BF16 = mybir.dt.bfloat16
AX = mybir.AxisListType
AF = mybir.ActivationFunctionType


def build_kernel():
    nc = bacc.Bacc("TRN2", target_bir_lowering=False, debug=False,
                   num_devices=N_CORES)

    feats = nc.dram_tensor("features", [B_LOC, T, D], F32, kind="ExternalInput")
    hidden = nc.dram_tensor("hidden", [B_LOC, D], F32, kind="ExternalInput")
    w1 = nc.dram_tensor("W1", [D, U], F32, kind="ExternalInput")
    w2 = nc.dram_tensor("W2", [D, U], F32, kind="ExternalInput")
    b1 = nc.dram_tensor("b1", [U], F32, kind="ExternalInput")
    b2 = nc.dram_tensor("b2", [U], F32, kind="ExternalInput")
    v = nc.dram_tensor("V", [U, 1], F32, kind="ExternalInput")

    ctx_out = nc.dram_tensor("ctx", [B_LOC, D], F32, kind="ExternalOutput")
    w_out = nc.dram_tensor("w", [B_LOC, T], F32, kind="ExternalOutput")

    with tile.TileContext(nc) as tc:
        with (
            tc.tile_pool(name="const", bufs=1) as cpool,
            tc.tile_pool(name="fb", bufs=10) as fb_pool,
            tc.tile_pool(name="ft", bufs=2) as ft_pool,
            tc.tile_pool(name="score", bufs=2) as sc_pool,
            tc.tile_pool(name="small", bufs=2) as sm_pool,
            tc.tile_pool(name="ps_tp", bufs=2, space="PSUM") as ps_tp,
            tc.tile_pool(name="ps_proj", bufs=3, space="PSUM") as ps_proj,
            tc.tile_pool(name="ps_small", bufs=2, space="PSUM") as ps_sm,
            tc.tile_pool(name="ps_misc", bufs=1, space="PSUM") as ps_misc,
        ):
            # ---- constants / weights in SBUF --------------------------------
            ident = cpool.tile([128, 128], BF16, tag="ident")
            make_identity(nc, ident)
            ident1 = cpool.tile([1, 1], F32, tag="ident1")
            nc.vector.memset(ident1[:], 1.0)
            ones_row = cpool.tile([1, 128], F32, tag="ones")
            nc.vector.memset(ones_row[:], 1.0)

            w1sb = cpool.tile([128, NJ, U], BF16, tag="w1")  # [dp, j, u]
            nc.gpsimd.dma_start(w1sb[:], w1.rearrange("(j p) u -> p j u", p=128))
            w2sb = cpool.tile([128, NJ, U], BF16, tag="w2")
            nc.gpsimd.dma_start(w2sb[:], w2.rearrange("(j p) u -> p j u", p=128))

            hT = cpool.tile([128, NJ, B_LOC], BF16, tag="ht")  # [dp, j, b]
            for j in range(NJ):
                nc.gpsimd.dma_start(
                    hT[:, j, :],
                    hidden[:, j * 128:(j + 1) * 128].rearrange("b p -> p b"),
                )

            vsb = cpool.tile([128, NM, 1], BF16, tag="v")  # [up, m, 1]
            nc.gpsimd.dma_start(vsb[:], v.rearrange("(m p) o -> p m o", p=128))

            b1T = cpool.tile([128, NM], F32, tag="b1")
            nc.gpsimd.dma_start(b1T[:], b1.rearrange("(m p) -> p m", p=128))
            b2T = cpool.tile([128, NM], F32, tag="b2")
            nc.gpsimd.dma_start(b2T[:], b2.rearrange("(m p) -> p m", p=128))
            b12T = cpool.tile([128, NM], F32, tag="b12")
            nc.vector.tensor_add(b12T[:], b1T[:], b2T[:])

            # ---- bh[u, m, b] = (hidden @ W2)^T + b1 + b2 --------------------
            bh = cpool.tile([128, NM, B_LOC], F32, tag="bh")
            for m in range(NM):
                ps = ps_proj.tile([128, CHUNK], F32, tag="proj")
                for j in range(NJ):
                    nc.tensor.matmul(
                        ps[:, :B_LOC],
                        lhsT=w2sb[:, j, m * 128:(m + 1) * 128],
                        rhs=hT[:, j, :],
                        start=(j == 0),
                        stop=(j == NJ - 1),
                    )
                nc.scalar.activation(bh[:, m, :], ps[:, :B_LOC], AF.Identity,
                                     bias=b12T[:, m:m + 1])

            # ---- main loop, software-pipelined over batches -----------------
            fb_tiles = {}  # (b, c) -> FB tile
            batch_state = {}  # b -> (ew_sb, s_inv)

            def main_phase_chunk(b, c):
                """Load + transpose + proj + tanh + logits for chunk c of batch b."""
                fb = fb_pool.tile([128, O_PER_CHUNK, D], BF16, tag="fb")
                nc.gpsimd.dma_start(
                    fb[:],
                    feats[b, c * CHUNK:(c + 1) * CHUNK, :]
                    .rearrange("(o p) d -> p o d", p=128),
                )
                fb_tiles[(b, c)] = fb

                # transpose FB -> FT  [dp, j, t(CHUNK)]
                ft = ft_pool.tile([128, NJ, CHUNK], BF16, tag="ft")
                for o in range(O_PER_CHUNK):
                    for jh in range(2):  # j groups of 4
                        ps = ps_tp.tile([128, 512], BF16, tag="tp")
                        for jj in range(4):
                            j = jh * 4 + jj
                            nc.tensor.transpose(
                                ps[:, jj * 128:(jj + 1) * 128],
                                fb[:, o, j * 128:(j + 1) * 128],
                                ident,
                            )
                        nc.any.tensor_copy(
                            out=ft[:, jh * 4:(jh + 1) * 4, o * 128:(o + 1) * 128],
                            in_=ps[:].rearrange("p (j t) -> p j t", j=4),
                        )

                # projT (per u-tile) -> tanh -> scoreT; then logits
                score = sc_pool.tile([128, NM, CHUNK], BF16, tag="score")
                for m in range(NM):
                    ps = ps_proj.tile([128, CHUNK], F32, tag="proj")
                    for j in range(NJ):
                        nc.tensor.matmul(
                            ps[:],
                            lhsT=w1sb[:, j, m * 128:(m + 1) * 128],
                            rhs=ft[:, j, :],
                            start=(j == 0),
                            stop=(j == NJ - 1),
                        )
                    nc.scalar.activation(score[:, m, :], ps[:], AF.Tanh,
                                         bias=bh[:, m, b:b + 1])

                psl = ps_sm.tile([1, CHUNK], F32, tag="lg")
                for m in range(NM):
                    nc.tensor.matmul(
                        psl[:],
                        lhsT=vsb[:, m, :],
                        rhs=score[:, m, :],
                        start=(m == 0),
                        stop=(m == NM - 1),
                    )
                return psl

            def batch_softmax(b, logits_psums):
                """Gather logits, softmax pieces, weights output, ewT layout."""
                la = sm_pool.tile([1, T], F32, tag="la")
                for c, psl in enumerate(logits_psums):
                    nc.any.tensor_copy(out=la[:, c * CHUNK:(c + 1) * CHUNK],
                                       in_=psl[:])
                mx = sm_pool.tile([1, 1], F32, tag="mx")
                nc.vector.reduce_max(mx[:], la[:], axis=AX.X)
                negmx = sm_pool.tile([1, 1], F32, tag="negmx")
                nc.vector.tensor_scalar_mul(negmx[:], mx[:], -1.0)

                ew = sm_pool.tile([1, T], F32, tag="ew")
                ssum = sm_pool.tile([1, 1], F32, tag="ssum")
                nc.scalar.activation(ew[:], la[:], AF.Exp, bias=negmx[:, 0:1],
                                     accum_out=ssum[:])
                s_inv = sm_pool.tile([1, 1], F32, tag="sinv")
                nc.vector.reciprocal(s_inv[:], ssum[:])

                # attention weights output (normalized)
                wn = sm_pool.tile([1, T], F32, tag="wn")
                nc.vector.tensor_scalar_mul(wn[:], ew[:], s_inv[:, 0:1])
                nc.sync.dma_start(w_out[b:b + 1, :], wn[:])

                # -max broadcast to 128 partitions via K=1 ones-matmul
                psb = ps_misc.tile([128, T // 128], F32, tag="laT")
                nc.tensor.matmul(psb[:, 0:1], lhsT=ones_row[:], rhs=negmx[:],
                                 start=True, stop=True)
                nmx_b = sm_pool.tile([128, 1], F32, tag="nmxb")
                nc.any.tensor_copy(out=nmx_b[:], in_=psb[:, 0:1])

                # logits -> t-partition layout via K=1 PE transposes, then exp
                pst = ps_misc.tile([128, T // 128], F32, tag="laT")
                for g in range(T // 128):
                    nc.tensor.transpose(pst[:, g:g + 1],
                                        la[0:1, g * 128:(g + 1) * 128], ident1)
                ewT = sm_pool.tile([128, T // 128], BF16, tag="ewt")
                nc.scalar.activation(ewT[:], pst[:], AF.Exp,
                                     bias=nmx_b[:, 0:1])
                batch_state[b] = (ewT, s_inv)

            def context_phase(b):
                """context[1, d] = sum_t ew[t] F[t, d] / s, via matmul."""
                ewT, s_inv = batch_state.pop(b)
                ctx_sb = sm_pool.tile([1, D], F32, tag="ctx")
                for h in range(2):
                    psc = ps_sm.tile([1, CHUNK], F32, tag="lg")
                    for g in range(T // 128):
                        c, o = divmod(g, O_PER_CHUNK)
                        nc.tensor.matmul(
                            psc[:],
                            lhsT=ewT[:, g:g + 1],
                            rhs=fb_tiles[(b, c)][:, o, h * 512:(h + 1) * 512],
                            start=(g == 0),
                            stop=(g == T // 128 - 1),
                        )
                    nc.vector.tensor_scalar_mul(
                        ctx_sb[:, h * 512:(h + 1) * 512], psc[:], s_inv[:, 0:1])
                nc.sync.dma_start(ctx_out[b:b + 1, :], ctx_sb[:])
                for c in range(N_CHUNKS):
                    del fb_tiles[(b, c)]

            for b in range(B_LOC):
                psls = []
                for c in range(N_CHUNKS):
                    psls.append(main_phase_chunk(b, c))
                    # after first chunk of batch b, run context of batch b-1
                    if c == 0 and b > 0:
                        context_phase(b - 1)
                batch_softmax(b, psls)
            context_phase(B_LOC - 1)

    nc.compile()
    return nc


_NC_CACHE = None


def _get_nc():
    global _NC_CACHE
    if _NC_CACHE is None:
        _NC_CACHE = build_kernel()
    return _NC_CACHE


def kernel(**inputs):
    feats = np.ascontiguousarray(np.asarray(inputs["features"], dtype=np.float32))
    hidden = np.ascontiguousarray(np.asarray(inputs["hidden"], dtype=np.float32))
    w1 = np.ascontiguousarray(np.asarray(inputs["W1"], dtype=np.float32))
    w2 = np.ascontiguousarray(np.asarray(inputs["W2"], dtype=np.float32))
    b1 = np.ascontiguousarray(np.asarray(inputs["b1"], dtype=np.float32))
    b2 = np.ascontiguousarray(np.asarray(inputs["b2"], dtype=np.float32))
    v = np.ascontiguousarray(np.asarray(inputs["V"], dtype=np.float32))

    nc = _get_nc()
    in_maps = []
    for i in range(N_CORES):
        sl = slice(i * B_LOC, (i + 1) * B_LOC)
        in_maps.append({
            "features": feats[sl],
            "hidden": hidden[sl],
            "W1": w1,
            "W2": w2,
            "b1": b1,
            "b2": b2,
            "V": v,
        })
    res = run_bass_kernel_spmd(nc, in_maps, core_ids=list(range(N_CORES)))

    ctx = np.concatenate([res.results[i]["ctx"] for i in range(N_CORES)], axis=0)
    w = np.concatenate([res.results[i]["w"] for i in range(N_CORES)], axis=0)
    return ctx, w.reshape(N_CORES * B_LOC, T, 1)


# revision 8
# speedup vs baseline: 1.0570x; 1.0570x over previous
"""Bahdanau attention kernel for Trainium2, SPMD over 8 NeuronCores.

Problem shapes: features [32, 2048, 1024] f32, hidden [32, 1024] f32,
W1/W2 [1024, 1024], b1/b2 [1024], V [1024, 1], bv [1].

Returns (context_vector [32, 1024] f32, attention_weights [32, 2048, 1] f32).

Sharding: data-parallel over batch B; each of the 8 cores handles 4 batches
end-to-end (no collectives needed).

Per-core pipeline, per batch b (T=2048 split into 4 chunks of 512 rows):
  1. SWDGE cast-DMA loads the F chunk f32->bf16 as FB [128(tp), 4(o), 1024(d)].
  2. PE transposes FB 128x128 blocks -> FT [128(dp), 8(j), 512(t)] (bf16).
  3. projT[u,t] = sum_j W1[dj,u].T @ FT[dj,t]  (bf16 matmul, PSUM f32).
  4. ScalarE tanh(projT + bh[u]) -> scoreT bf16, where bh = hidden@W2 + b1 + b2
     enters as the per-partition activation bias (free).
  5. logits[1,t] = sum_m V[um].T @ scoreT[um,t]  (matmul, M=1).
  6. Batch end: -max, exp (with accum_out sum), reciprocal; weights out;
     exp-weights bounced through DRAM into t-partition layout [128, 16] bf16;
     context[1,d] = sum_t ew[t] * FB[t,d] via matmul, scaled by 1/sum.
  Context phase of batch b is emitted after chunk 0 of batch b+1 so the PE
  never stalls on the softmax latency.
"""

import numpy as np

import concourse.bass as bass
import concourse.mybir as mybir
import concourse.tile as tile
from concourse import bacc
from concourse.bass_utils import run_bass_kernel_spmd

N_CORES = 8
B_LOC = 4  # batches per core
T = 2048
D = 1024
U = 1024
CHUNK = 512  # t rows per chunk
N_CHUNKS = T // CHUNK  # 4
O_PER_CHUNK = CHUNK // 128  # 4 t-subtiles per chunk
NJ = D // 128  # 8 d-tiles
NM = U // 128  # 8 u-tiles

F32 = mybir.dt.float32
BF16 = mybir.dt.bfloat16
AX = mybir.AxisListType
AF = mybir.ActivationFunctionType


def build_kernel():
    nc = bacc.Bacc("TRN2", target_bir_lowering=False, debug=False,
                   num_devices=N_CORES)

    feats = nc.dram_tensor("features", [B_LOC, T, D], F32, kind="ExternalInput")
    hidden = nc.dram_tensor("hidden", [B_LOC, D], F32, kind="ExternalInput")
    w1 = nc.dram_tensor("W1", [D, U], F32, kind="ExternalInput")
    w2 = nc.dram_tensor("W2", [D, U], F32, kind="ExternalInput")
    b1 = nc.dram_tensor("b1", [U], F32, kind="ExternalInput")
    b2 = nc.dram_tensor("b2", [U], F32, kind="ExternalInput")
    v = nc.dram_tensor("V", [U, 1], F32, kind="ExternalInput")

    ctx_out = nc.dram_tensor("ctx", [B_LOC, D], F32, kind="ExternalOutput")
    w_out = nc.dram_tensor("w", [B_LOC, T], F32, kind="ExternalOutput")

    with tile.TileContext(nc) as tc:
        with (
            tc.tile_pool(name="const", bufs=1) as cpool,
            tc.tile_pool(name="fb", bufs=10) as fb_pool,
            tc.tile_pool(name="ft", bufs=2) as ft_pool,
            tc.tile_pool(name="score", bufs=2) as sc_pool,
            tc.tile_pool(name="small", bufs=2) as sm_pool,
            tc.tile_pool(name="ps_proj", bufs=5, space="PSUM") as ps_proj,
            tc.tile_pool(name="ps_small", bufs=2, space="PSUM") as ps_sm,
            tc.tile_pool(name="ps_misc", bufs=1, space="PSUM") as ps_misc,
        ):
            # ---- constants / weights in SBUF --------------------------------
            ident1 = cpool.tile([1, 1], F32, tag="ident1")
            nc.vector.memset(ident1[:], 1.0)
            ones_row = cpool.tile([1, 128], F32, tag="ones")
            nc.vector.memset(ones_row[:], 1.0)

            w1sb = cpool.tile([128, NJ, U], BF16, tag="w1")  # [dp, j, u]
            nc.gpsimd.dma_start(w1sb[:], w1.rearrange("(j p) u -> p j u", p=128))
            w2sb = cpool.tile([128, NJ, U], BF16, tag="w2")
            nc.gpsimd.dma_start(w2sb[:], w2.rearrange("(j p) u -> p j u", p=128))

            hT = cpool.tile([128, NJ, B_LOC], BF16, tag="ht")  # [dp, j, b]
            for j in range(NJ):
                nc.gpsimd.dma_start(
                    hT[:, j, :],
                    hidden[:, j * 128:(j + 1) * 128].rearrange("b p -> p b"),
                )

            vsb = cpool.tile([128, NM, 1], BF16, tag="v")  # [up, m, 1]
            nc.gpsimd.dma_start(vsb[:], v.rearrange("(m p) o -> p m o", p=128))

            b1T = cpool.tile([128, NM], F32, tag="b1")
            nc.gpsimd.dma_start(b1T[:], b1.rearrange("(m p) -> p m", p=128))
            b2T = cpool.tile([128, NM], F32, tag="b2")
            nc.gpsimd.dma_start(b2T[:], b2.rearrange("(m p) -> p m", p=128))
            b12T = cpool.tile([128, NM], F32, tag="b12")
            nc.vector.tensor_add(b12T[:], b1T[:], b2T[:])

            # ---- bh[u, m, b] = (hidden @ W2)^T + b1 + b2 --------------------
            bh = cpool.tile([128, NM, B_LOC], F32, tag="bh")
            for m in range(NM):
                ps = ps_proj.tile([128, CHUNK], F32, tag="proj")
                for j in range(NJ):
                    nc.tensor.matmul(
                        ps[:, :B_LOC],
                        lhsT=w2sb[:, j, m * 128:(m + 1) * 128],
                        rhs=hT[:, j, :],
                        start=(j == 0),
                        stop=(j == NJ - 1),
                    )
                nc.scalar.activation(bh[:, m, :], ps[:, :B_LOC], AF.Identity,
                                     bias=b12T[:, m:m + 1])

            # ---- main loop, software-pipelined over batches -----------------
            fb_tiles = {}  # (b, c) -> FB tile
            batch_state = {}  # b -> (ew_sb, s_inv)

            def main_phase_chunk(b, c):
                """Load + transpose + proj + tanh + logits for chunk c of batch b."""
                fb = fb_pool.tile([128, O_PER_CHUNK, D], BF16, tag="fb")
                nc.gpsimd.dma_start(
                    fb[:],
                    feats[b, c * CHUNK:(c + 1) * CHUNK, :]
                    .rearrange("(o p) d -> p o d", p=128),
                )
                fb_tiles[(b, c)] = fb

                # transpose FB -> FT via DMA xbar: ft[p, o, j, c] = FT[j*128+p, o*128+c]
                ft = ft_pool.tile([128, O_PER_CHUNK, NJ, 128], BF16, tag="ft")
                nc.sync.dma_start_transpose(
                    ft[:].rearrange("p o j c -> p (o j) c"),
                    fb[:].rearrange("p o d -> p (o d)"),
                )

                # projT (per u-tile) -> tanh -> scoreT; then logits
                score = sc_pool.tile([128, NM, CHUNK], BF16, tag="score")
                for m in range(NM):
                    ps = ps_proj.tile([128, CHUNK], F32, tag="proj")
                    for j in range(NJ):
                        nc.tensor.matmul(
                            ps[:],
                            lhsT=w1sb[:, j, m * 128:(m + 1) * 128],
                            rhs=ft[:, :, j, :],
                            start=(j == 0),
                            stop=(j == NJ - 1),
                        )
                    nc.scalar.activation(score[:, m, :], ps[:], AF.Tanh,
                                         bias=bh[:, m, b:b + 1])

                psl = ps_sm.tile([1, CHUNK], F32, tag="lg")
                for m in range(NM):
                    nc.tensor.matmul(
                        psl[:],
                        lhsT=vsb[:, m, :],
                        rhs=score[:, m, :],
                        start=(m == 0),
                        stop=(m == NM - 1),
                    )
                return psl

            def batch_softmax(b, logits_psums):
                """Gather logits, softmax pieces, weights output, ewT layout."""
                la = sm_pool.tile([1, T], F32, tag="la")
                for c, psl in enumerate(logits_psums):
                    nc.any.tensor_copy(out=la[:, c * CHUNK:(c + 1) * CHUNK],
                                       in_=psl[:])
                mx = sm_pool.tile([1, 1], F32, tag="mx")
                nc.vector.reduce_max(mx[:], la[:], axis=AX.X)
                negmx = sm_pool.tile([1, 1], F32, tag="negmx")
                nc.vector.tensor_scalar_mul(negmx[:], mx[:], -1.0)

                ew = sm_pool.tile([1, T], F32, tag="ew")
                ssum = sm_pool.tile([1, 1], F32, tag="ssum")
                nc.scalar.activation(ew[:], la[:], AF.Exp, bias=negmx[:, 0:1],
                                     accum_out=ssum[:])
                s_inv = sm_pool.tile([1, 1], F32, tag="sinv")
                nc.vector.reciprocal(s_inv[:], ssum[:])

                # attention weights output (normalized)
                wn = sm_pool.tile([1, T], F32, tag="wn")
                nc.vector.tensor_scalar_mul(wn[:], ew[:], s_inv[:, 0:1])
                nc.sync.dma_start(w_out[b:b + 1, :], wn[:])

                # -max broadcast to 128 partitions via K=1 ones-matmul
                psb = ps_misc.tile([128, T // 128], F32, tag="laT")
                nc.tensor.matmul(psb[:, 0:1], lhsT=ones_row[:], rhs=negmx[:],
                                 start=True, stop=True)
                nmx_b = sm_pool.tile([128, 1], F32, tag="nmxb")
                nc.any.tensor_copy(out=nmx_b[:], in_=psb[:, 0:1])

                # logits -> t-partition layout via K=1 PE transposes, then exp
                pst = ps_misc.tile([128, T // 128], F32, tag="laT")
                for g in range(T // 128):
                    nc.tensor.transpose(pst[:, g:g + 1],
                                        la[0:1, g * 128:(g + 1) * 128], ident1)
                ewT = sm_pool.tile([128, T // 128], BF16, tag="ewt")
                nc.scalar.activation(ewT[:], pst[:], AF.Exp,
                                     bias=nmx_b[:, 0:1])
                batch_state[b] = (ewT, s_inv)

            def context_phase(b):
                """context[1, d] = sum_t ew[t] F[t, d] / s, via matmul."""
                ewT, s_inv = batch_state.pop(b)
                ctx_sb = sm_pool.tile([1, D], F32, tag="ctx")
                for h in range(2):
                    psc = ps_sm.tile([1, CHUNK], F32, tag="lg")
                    for g in range(T // 128):
                        c, o = divmod(g, O_PER_CHUNK)
                        nc.tensor.matmul(
                            psc[:],
                            lhsT=ewT[:, g:g + 1],
                            rhs=fb_tiles[(b, c)][:, o, h * 512:(h + 1) * 512],
                            start=(g == 0),
                            stop=(g == T // 128 - 1),
                        )
                    nc.vector.tensor_scalar_mul(
                        ctx_sb[:, h * 512:(h + 1) * 512], psc[:], s_inv[:, 0:1])
                nc.sync.dma_start(ctx_out[b:b + 1, :], ctx_sb[:])
                for c in range(N_CHUNKS):
                    del fb_tiles[(b, c)]

            for b in range(B_LOC):
                psls = []
                for c in range(N_CHUNKS):
                    psls.append(main_phase_chunk(b, c))
                    # after first chunk of batch b, run context of batch b-1
                    if c == 0 and b > 0:
                        context_phase(b - 1)
                batch_softmax(b, psls)
            context_phase(B_LOC - 1)

    nc.compile()
    return nc


_NC_CACHE = None


def _get_nc():
    global _NC_CACHE
    if _NC_CACHE is None:
        _NC_CACHE = build_kernel()
    return _NC_CACHE


def kernel(**inputs):
    feats = np.ascontiguousarray(np.asarray(inputs["features"], dtype=np.float32))
    hidden = np.ascontiguousarray(np.asarray(inputs["hidden"], dtype=np.float32))
    w1 = np.ascontiguousarray(np.asarray(inputs["W1"], dtype=np.float32))
    w2 = np.ascontiguousarray(np.asarray(inputs["W2"], dtype=np.float32))
    b1 = np.ascontiguousarray(np.asarray(inputs["b1"], dtype=np.float32))
    b2 = np.ascontiguousarray(np.asarray(inputs["b2"], dtype=np.float32))
    v = np.ascontiguousarray(np.asarray(inputs["V"], dtype=np.float32))

    nc = _get_nc()
    in_maps = []
    for i in range(N_CORES):
        sl = slice(i * B_LOC, (i + 1) * B_LOC)
        in_maps.append({
            "features": feats[sl],
            "hidden": hidden[sl],
            "W1": w1,
            "W2": w2,
            "b1": b1,
            "b2": b2,
            "V": v,
        })
    res = run_bass_kernel_spmd(nc, in_maps, core_ids=list(range(N_CORES)))

    ctx = np.concatenate([res.results[i]["ctx"] for i in range(N_CORES)], axis=0)
    w = np.concatenate([res.results[i]["w"] for i in range(N_CORES)], axis=0)
    return ctx, w.reshape(N_CORES * B_LOC, T, 1)
